# revision 2
# baseline (speedup 1.0000x reference)
"""Trainium2 kernel for nn_BlockLinear: gather -> per-block GEMM -> scatter-add.

Key insight: the whole op is linear in x, so gather/einsum/scatter fold into a
single dense GEMM  out[t, o] = sum_k x[t, k] * Wfull[k, o] + bias[o]  where
Wfull[k, o] = sum_{n,i,j} [input_indices[n,i]==k][output_indices[n,j]==o] * W[n,j,i].

Wfull is built on host (bincount scatter-add, exact fp64 accumulation), then the
GEMM runs on 8 NeuronCores, sharded 2D: 4 token groups x 2 out-feature groups.
Matmuls use the fp32r dtype (fp32 with 11-bit mantissa, 4x faster than fp32 on
the PE); inputs are pre-rounded on host to valid fp32r values.
"""

import numpy as np
import concourse.bacc as bacc
import concourse.mybir as mybir
import concourse.tile as tile
from concourse.bass_utils import run_bass_kernel_spmd

# problem shapes (hardcoded per contract)
B, S = 2, 2048
IN_FEATURES = 4096
OUT_FEATURES = 4096
NTOKENS = B * S                  # 4096

NCORES = 8
TG, OG = 4, 2                    # token groups x out-feature groups
T = NTOKENS // TG                # 1024 tokens per core
O = OUT_FEATURES // OG           # 2048 out features per core
P = 128
KT = IN_FEATURES // P            # 32 contraction tiles
OT = O // P                      # 16 out-feature tiles per core
NTOK = 512                       # moving free dim per matmul
TB = T // NTOK                   # 2 token blocks per core

F32R = mybir.dt.float32r
F32 = mybir.dt.float32

# knobs for test.py
TRACE = False
LAST_RESULTS = None


def round_fp32r(a: np.ndarray) -> np.ndarray:
    """Round fp32 to the nearest fp32r-representable value (11-bit mantissa)."""
    u = np.ascontiguousarray(a, dtype=np.float32).view(np.uint32)
    r = (u.astype(np.uint64) + 0x7FF + ((u >> 12) & 1)) & 0xFFFFF000
    return r.astype(np.uint32).view(np.float32)


def build_nc():
    nc = bacc.Bacc()
    # per k-slab: [xT tb0 | xT tb1 | W col for o-tile 0]
    xw = nc.dram_tensor("xw", [KT, P, TB * NTOK + P], F32R, kind="ExternalInput")
    # W for o-tiles >= 1
    wrest = nc.dram_tensor("wrest", [OT - 1, KT, P, P], F32R, kind="ExternalInput")
    # [bias (O) | ones (NTOK)]
    bo = nc.dram_tensor("bo", [1, O + NTOK], F32R, kind="ExternalInput")
    out = nc.dram_tensor("out", [OT, TB, P, NTOK], F32, kind="ExternalOutput")

    with tile.TileContext(nc) as tc:
        with (
            tc.tile_pool(name="xw_sb", bufs=1) as xw_sb,
            tc.tile_pool(name="w_sb", bufs=8) as w_sb,
            tc.tile_pool(name="o_sb", bufs=6) as o_sb,
            tc.tile_pool(name="ps", bufs=4, space="PSUM") as ps,
        ):
            bo_t = xw_sb.tile([1, O + NTOK], F32R, tag="bo")
            nc.sync.dma_start(out=bo_t, in_=bo[:, :])
            xw_t = []
            for k in range(KT):
                t = xw_sb.tile([P, TB * NTOK + P], F32R, tag=f"xw{k}")
                nc.sync.dma_start(out=t, in_=xw[k])
                xw_t.append(t)

            for o in range(OT):
                psums = [
                    ps.tile([P, NTOK], F32, tag=f"ps{tb}", name=f"ps_{o}_{tb}")
                    for tb in range(TB)
                ]
                for tb in range(TB):
                    # bias matmul: psum[m, n] = bias[o*128+m]; also carries the
                    # psum-bank-free wait so later matmuls keep <=1 sync wait
                    nc.tensor.matmul(
                        psums[tb],
                        bo_t[:, o * P : (o + 1) * P],
                        bo_t[:, O : O + NTOK],
                        start=True,
                        stop=False,
                    )
                for k in range(KT):
                    if o == 0:
                        lhsT = xw_t[k][:, TB * NTOK :]
                    else:
                        wt = w_sb.tile([P, P], F32R)
                        nc.sync.dma_start(out=wt, in_=wrest[o - 1, k])
                        lhsT = wt
                    for tb in range(TB):
                        nc.tensor.matmul(
                            psums[tb],
                            lhsT,
                            xw_t[k][:, tb * NTOK : (tb + 1) * NTOK],
                            start=False,
                            stop=(k == KT - 1),
                        )
                for tb in range(TB):
                    o_t = o_sb.tile([P, NTOK], F32)
                    nc.vector.tensor_copy(out=o_t, in_=psums[tb])
                    nc.sync.dma_start(out=out[o, tb], in_=o_t)
    nc.finalize()
    return nc


_NC = None


def _get_nc():
    global _NC
    if _NC is None:
        _NC = build_nc()
    return _NC


def _build_wfull(weights, input_indices, output_indices):
    """Wfull[k, o] = sum over blocks/dups of weights[n, j, i]."""
    ii = np.asarray(input_indices).astype(np.int64)     # [NBLK, BI]
    oi = np.asarray(output_indices).astype(np.int64)    # [NBLK, BO]
    w = np.asarray(weights, dtype=np.float64)           # [NBLK, BO, BI]
    flat = (ii[:, :, None] * OUT_FEATURES + oi[:, None, :]).ravel()  # [n, i, j]
    vals = np.ascontiguousarray(np.swapaxes(w, 1, 2)).ravel()        # [n, i, j]
    wfull = np.bincount(flat, weights=vals, minlength=IN_FEATURES * OUT_FEATURES)
    return wfull.reshape(IN_FEATURES, OUT_FEATURES).astype(np.float32)


def kernel(x, weights, bias, input_indices, output_indices):
    global LAST_RESULTS
    x = np.asarray(x, dtype=np.float32)
    bias = np.asarray(bias, dtype=np.float32)

    wfull = round_fp32r(_build_wfull(weights, input_indices, output_indices))
    xr = round_fp32r(x.reshape(NTOKENS, IN_FEATURES))
    biasr = round_fp32r(bias)
    ones = np.ones((NTOK,), dtype=np.float32)

    in_maps = []
    for c in range(NCORES):
        tg, og = divmod(c, OG)
        xT = np.ascontiguousarray(xr[tg * T : (tg + 1) * T, :].T)   # [K, T]
        xw = np.empty((KT, P, TB * NTOK + P), np.float32)
        xw[:, :, : TB * NTOK] = xT.reshape(KT, P, T)
        xw[:, :, TB * NTOK :] = wfull[:, og * O : og * O + P].reshape(KT, P, P)
        wr = np.ascontiguousarray(
            wfull[:, og * O + P : (og + 1) * O]
            .reshape(KT, P, OT - 1, P)
            .transpose(2, 0, 1, 3)
        )
        bo = np.concatenate([biasr[og * O : (og + 1) * O], ones]).reshape(1, -1)
        in_maps.append({"xw": xw, "wrest": wr, "bo": bo})

    nc = _get_nc()
    res = run_bass_kernel_spmd(nc, in_maps, list(range(NCORES)))
    LAST_RESULTS = res

    full = np.empty((NTOKENS, OUT_FEATURES), np.float32)
    for c in range(NCORES):
        tg, og = divmod(c, OG)
        o4 = res.results[c]["out"]                       # [OT, TB, P, NTOK]
        blk = o4.transpose(1, 3, 0, 2).reshape(T, O)     # [t, o]
        full[tg * T : (tg + 1) * T, og * O : (og + 1) * O] = blk
    return full.reshape(B, S, OUT_FEATURES)
